# revision 29
# baseline (speedup 1.0000x reference)
"""Trainium2 Bass kernel for nn_ManifoldDynamic (v2, latency-optimized).

Math (per sample b):
    f = tanh(x@Wf1.T + bf1)@Wf2.T + bf2        (same for g, k)
    dx = f + g
    Jf = Wk1.T @ (S * (Wk2.T @ f)),  S = 1 - tanh(h_k)^2   (JVP, no Jacobian)
    c1 > EPS  <=>  ||Jf||^2 > 3600*(||k||^2)^9
    c2 < -EPS <=>  <k,JG> + EPS < 20*(||k||^2)^5
    out = dx * (1 - 0.5*mask)

Design notes (single-iteration latency is the metric; the timing harness
wraps the body in a For_i whose back edge has an all-engine barrier, so
iterations serialize and the body's serial latency is what counts):
  - All L2 matmuls emit TRANSPOSED outputs [n-part, (nblk, b)] so the JVP
    consumes them directly -- no PE transposes, no [16,256]-layout vector
    ops.  The output stays transposed; the host untransposes for free.
  - Sample-dim reductions (||k||^2, ||Jf||^2, <k,JG>) are ones-vector
    matmuls contracting the 128 partitions; the mask scalar chain runs on
    [1,16] tiles; fac is partition-broadcast by one more rank-1 matmul.
  - f/g weights stream in fp16 (2MB, accuracy-bound: fp8 gives 5e-2 >
    the 2e-2 gate); the whole k/JVP path is fp8 (1MB) -- the mask
    comparisons have ~1e10 of margin.
  - HW measurement showed DMA transfers serialize per core (~3ns/KB,
    ~340GB/s, independent of queue count) with ~1.1us fixed cost per
    transfer, so the whole weight set ships as THREE chunks: one 2MB
    f16 blob (biases | x.T | w1fg | w2fg) on SP, then w1k|w2k and
    wk2|wk1 (fp8) on ACT.  Compute is emitted in chunk-arrival order;
    only V/J matmuls + the mask chain + the output DMA trail the
    stream.  The tanh ACT table is preloaded by a dummy activation.
"""

import numpy as np
import ml_dtypes

import concourse.bass as bass
import concourse.mybir as mybir
from concourse.tile import TileContext
from concourse.vector_clock import ScopedClock
from concourse.bass_utils import run_bass_kernel_spmd

N_CORES = 8
GP_COPY = True       # casts on gpsimd; False: DVE
LEVEL = 99           # phase truncation for profiling: 0=DMA only, 2=no JVP
BS, N, H = 128, 256, 1024
B = BS // N_CORES          # 16 rows per core
NB = N // 128              # 2 n-blocks
HB = H // 128              # 8 h-blocks
ALPHA, BETA, EPS = 60.0, 20.0, 1e-8

F16 = mybir.dt.float16
F32 = mybir.dt.float32
F8 = mybir.dt.float8e4
NP8 = ml_dtypes.float8_e4m3
ALU = mybir.AluOpType
ACTF = mybir.ActivationFunctionType

# bias tensor (f32, [128, 30]): col j = bias slice for region j, values on
# the partition axis (h-in-block for L1 slices, n-in-block for L2 slices)
C_B1F = 0            # b1f: cols 0-7
C_B1G = 8            # b1g: cols 8-15
C_B1K = 16           # b1k: cols 16-23
C_B2F = 24           # b2f: cols 24-25
C_B2G = 26           # b2g: cols 26-27
C_B2K = 28           # b2k: cols 28-29
C_XT = 30            # x.T packed, 32 cols f32 (cast on chip)
C_TOT = 62


class PatchedTileContext(TileContext):
    """walrus in this env rejects >1 sync wait per instruction; after
    scheduling, hoist excess waits onto same-engine NOPs placed directly
    before the instruction (same gating, one wait per instruction)."""

    _ws_counter = 0

    def _split_waits(self):
        import bass_rust as _br

        nc = self.nc
        for fn in nc.m.functions:
            for blk in fn.blocks:
                insts = list(blk.instructions)
                out = []
                changed = False
                for inst in insts:
                    si = inst.sync_info
                    if si is not None and si.on_wait and len(si.on_wait) > 1:
                        waits = list(si.on_wait)
                        del si.on_wait[:]
                        si.on_wait.append(waits[-1])
                        for w in waits[:-1]:
                            PatchedTileContext._ws_counter += 1
                            nop = _br.InstNoOp(
                                name=f"waitsplit_{PatchedTileContext._ws_counter}"
                            )
                            nop.engine = inst.engine
                            nop.sync_info = mybir.SyncInfo(
                                on_wait=[w], on_update=[])
                            nc.register_instruction(nop)
                            out.append(nop)
                        changed = True
                    out.append(inst)
                if changed:
                    blk.instructions = out

    def _drain_and_barrier(self, tick_clock, wait_clock):
        drain_inst = self.nc.sync.drain()
        wait_clock.add_sem_waits(
            drain_inst.ins, ScopedClock({None: tick_clock.global_clock})
        )
        self.nc.all_engine_barrier()
        assert self.sems is not None
        popped = self.nc._tile_sem_poison_stack.pop()
        assert popped is self._sem_poison
        self.nc.clear_and_free_semaphores(list(self.sems.allocated().values()))
        self.nc.all_engine_barrier()
        self._split_waits()
        # SWDGE descriptor-gen emits extended InstISA ops; inside a For_i
        # their .instr bytes are not populated by the default raw-bass
        # path -> "ISA wrong length" in codegen.  Lower them here.
        mybir.codegen_inst_isa_subclasses(self.nc)


def _pack(arr, pblk):
    """[pblk*128, F] -> [128, pblk*F]: partition-block p of the original
    lands at free-dim columns [p*F, (p+1)*F)."""
    k, f = arr.shape
    assert k == pblk * 128
    return np.ascontiguousarray(
        arr.reshape(pblk, 128, f).transpose(1, 0, 2).reshape(128, pblk * f)
    )


def _pack_l1(w1T, hblk):
    """[256, hblk*128] -> [128, hblk*2*128] as (hblk, kblk) [128,128]
    tiles: tile (i, k) at columns (i*2+k)*128."""
    return np.ascontiguousarray(
        w1T.reshape(NB, 128, hblk, 128).transpose(1, 2, 0, 3)
        .reshape(128, hblk * NB * 128)
    )


def declare_io(nc):
    def din(name, shape, dt):
        return nc.dram_tensor(name, shape, dt, kind="ExternalInput").ap()

    io = dict(
        wfg_d=din("wfg", [128, 64 + 8192], F16),  # bias|x.T|w1fg|w2fg
        ka_d=din("ka", [128, 6144], F8),       # w1k | w2k | wk2
        wk1_d=din("wk1", [128, 2048], F8),     # Wk1 natural (J), last chunk
        y_d=nc.dram_tensor("y", [128, NB * B], F32, kind="ExternalOutput").ap(),
    )
    return io


def emit_body(nc, tc, wfg_d, ka_d, wk1_d, y_d):
    with (
        tc.tile_pool(name="wpool", bufs=1) as wp,
        tc.tile_pool(name="apool", bufs=1) as ap,
        tc.tile_pool(name="psum", bufs=1, space="PSUM") as pp,
    ):
        # ---------------- SBUF tiles --------------------------------
        wfg = wp.tile([128, 64 + 8192], F16, tag="wfg")
        ka = wp.tile([128, 6144], F8, tag="ka")
        wk1t = wp.tile([128, 2048], F8, tag="wk1")
        cst = wfg[:, 0:C_TOT]
        xt16 = wfg[:, C_XT:C_XT + NB * B]
        w1fg = wfg[:, 64:64 + 4096]
        w2fg = wfg[:, 64 + 4096:64 + 8192]
        w1k = ka[:, 0:2048]
        w2k = ka[:, 2048:4096]
        wk2 = ka[:, 4096:6144]
        wk1 = wk1t[:, :]

        ones128 = ap.tile([1, 128], F16, tag="ones128")
        onescol = ap.tile([128, 1], F32, tag="onescol")
        tscr = ap.tile([1, B], F16, tag="tscr")
        xt8 = ap.tile([128, NB * B], F8, tag="xt8")
        aTf = ap.tile([128, HB * B], F16, tag="aTf")
        aTg = ap.tile([128, HB * B], F16, tag="aTg")
        aTk = ap.tile([128, HB * B], F8, tag="aTk")
        sq = ap.tile([128, HB * B], F32, tag="sq")
        sT = ap.tile([128, HB * B], F32, tag="sT")
        fT_sb = ap.tile([128, NB * B], F32, tag="fT_sb")
        gT_sb = ap.tile([128, NB * B], F32, tag="gT_sb")
        kT_sb = ap.tile([128, NB * B], F32, tag="kT_sb")
        fg8 = ap.tile([128, NB * 2 * B], F8, tag="fg8")   # (nb, c, b)
        svt = ap.tile([128, HB * 2 * B], F8, tag="svt")   # (j, c, b)
        dxT = ap.tile([128, NB * B], F32, tag="dxT")
        kk = ap.tile([128, NB * B], F32, tag="kk")
        jfsq = ap.tile([128, NB * B], F32, tag="jfsq")
        kjg = ap.tile([128, NB * B], F32, tag="kjg")
        s2sb = ap.tile([1, B], F32, tag="s2sb")
        p4 = ap.tile([1, B], F32, tag="p4")
        p8 = ap.tile([1, B], F32, tag="p8")
        p16 = ap.tile([1, B], F32, tag="p16")
        t1 = ap.tile([1, B], F32, tag="t1")
        t2 = ap.tile([1, B], F32, tag="t2")
        m1 = ap.tile([1, B], F32, tag="m1")
        m2 = ap.tile([1, B], F32, tag="m2")
        mm = ap.tile([1, B], F32, tag="mm")
        fac16 = ap.tile([1, B], F16, tag="fac16")
        outT = ap.tile([128, NB * B], F32, tag="outT")

        # ---------------- PSUM tiles (8 banks) ----------------------
        hTf = pp.tile([128, HB * B], F32, tag="hTf")
        hTg = pp.tile([128, HB * B], F32, tag="hTg")
        hTk = pp.tile([128, HB * B], F32, tag="hTk")
        vt = pp.tile([128, HB * 2 * B], F32, tag="vt")    # (j, c, b)
        l2ps = pp.tile([128, 3 * NB * B], F32, tag="l2ps")  # f | k | g
        jT = pp.tile([128, NB * 2 * B], F32, tag="jT")    # (nb, c, b)
        facBC = pp.tile([128, B], F32, tag="facBC")
        red = pp.tile([1, 3 * B], F32, tag="red")         # s2 | jf2 | c2dot

        fT_ps = l2ps[:, 0:NB * B]
        kT_ps = l2ps[:, NB * B:2 * NB * B]
        gT_ps = l2ps[:, 2 * NB * B:3 * NB * B]

        # ---------------- DMA issues --------------------------------
        # SP: x + consts (tiny), then the f weights.  DVE HWDGE carries
        # the g weights in parallel; ACT HWDGE the k forward weights;
        # Pool SWDGE the JVP weights.
        HALF1 = 16 * NB * 128 // 2
        HALF2 = HB * 2 * N // 2
        # Measured on HW: DMA transfers serialize chip-side (~3ns/KB
        # aggregate) and each chunk carries ~1.1us of fixed overhead, so
        # FEWER, BIGGER chunks win.  4 chunks, in compute order; the k
        # JVP weights land last (only J + mask + out trail the stream).
        nc.sync.dma_start(wfg[:], wfg_d[:])
        nc.scalar.dma_start(ka[:], ka_d[:])
        nc.scalar.dma_start(wk1t[:], wk1_d[:])
        gcopy = nc.gpsimd.tensor_copy if GP_COPY else nc.vector.tensor_copy
        nc.vector.memset(ones128[:], 1.0)
        nc.vector.memset(onescol[:], 1.0)
        gcopy(xt8[:], xt16)
        # Preload the tanh table set during the DMA stream: a throwaway
        # ACTIVATE pays the ~1.4us ACT_TABLE_LOAD off the critical path.
        nc.scalar.activation(tscr[:], ones128[0:1, 0:B], ACTF.Tanh)

        # ---------------- matmul helpers ----------------------------
        def l1_mlp(hT, w, xw, bc0):
            """L1 matmuls + in-place PSUM bias add (broadcast over b)."""
            for j in range(HB):
                for nk in range(NB):
                    nc.tensor.matmul(
                        hT[:, j * B:(j + 1) * B],
                        w[:, (j * NB + nk) * 128:(j * NB + nk + 1) * 128],
                        xw[:, nk * B:(nk + 1) * B],
                        start=(nk == 0), stop=(nk == NB - 1),
                    )
            hT_v = hT[:, :].rearrange("p (j b) -> p j b", b=B)
            b_bc = (cst[:, bc0:bc0 + HB].unsqueeze(2)
                    .broadcast_to((128, HB, B)))
            nc.vector.tensor_tensor(hT_v, hT_v, b_bc, ALU.add)

        def l2_mlp(ps, w, wcol, aT):
            for nb in range(NB):
                for j in range(HB):
                    nc.tensor.matmul(
                        ps[:, nb * B:(nb + 1) * B],
                        w[:, wcol + j * N + nb * 128:
                          wcol + j * N + nb * 128 + 128],
                        aT[:, j * B:(j + 1) * B],
                        start=(j == 0), stop=(j == HB - 1),
                    )

        def l2_bias_move(dst, ps, bc0):
            """PSUM -> SBUF with the L2 bias folded into the move."""
            dst_v = dst[:].rearrange("p (nb b) -> p nb b", b=B)
            ps_v = ps.rearrange("p (nb b) -> p nb b", b=B)
            b_bc = (cst[:, bc0:bc0 + NB].unsqueeze(2)
                    .broadcast_to((128, NB, B)))
            nc.vector.tensor_tensor(dst_v, ps_v, b_bc, ALU.add)

        # ============ interleaved program (emission order = dep order;
        # per-engine execution order is the subsequence per engine) =====
        if LEVEL < 1:
            # consumers force every DMA into the iteration
            for tl_ in (wfg, ka, wk1t):
                nc.vector.tensor_copy(outT[0:1, 0:B],
                                      tl_[0:1, tl_.shape[1] - B:])
            nc.vector.tensor_copy(outT[:], cst[:, 0:NB * B])
            nc.sync.dma_start(y_d[:], outT[:])
            return
        # f and g chains complete first (their weights arrive in chunk 2)
        fg8_v = fg8[:].rearrange("p (nb c b) -> p nb c b", c=2, b=B)
        l1_mlp(hTf, w1fg, xt16, C_B1F)
        nc.scalar.activation(aTf[:], hTf[:], ACTF.Tanh)
        l1_mlp(hTg, w1fg[:, HALF1:], xt16, C_B1G)
        nc.scalar.activation(aTg[:], hTg[:], ACTF.Tanh)
        l2_mlp(fT_ps, w2fg, 0, aTf)
        l2_bias_move(fT_sb, fT_ps, C_B2F)
        gcopy(
            fg8_v[:, :, 0, :],
            fT_sb[:].rearrange("p (nb b) -> p nb b", b=B))
        l2_mlp(gT_ps, w2fg, HB * N, aTg)
        l2_bias_move(gT_sb, gT_ps, C_B2G)
        gcopy(
            fg8_v[:, :, 1, :],
            gT_sb[:].rearrange("p (nb b) -> p nb b", b=B))
        nc.vector.tensor_tensor(dxT[:], fT_sb[:], gT_sb[:], ALU.add)
        # k chain (chunk ka); wk2 lands with the same chunk
        l1_mlp(hTk, w1k, xt8, C_B1K)
        nc.scalar.activation(aTk[:], hTk[:], ACTF.Tanh)
        nc.scalar.square(sq[:], aTk[:])
        nc.vector.tensor_scalar(sT[:], sq[:], -1.0, 1.0, ALU.mult, ALU.add)
        l2_mlp(kT_ps, w2k, 0, aTk)
        l2_bias_move(kT_sb, kT_ps, C_B2K)
        nc.vector.tensor_tensor(kk[:], kT_sb[:], kT_sb[:], ALU.mult)
        if LEVEL < 3:
            nc.vector.tensor_copy(outT[0:1, 0:B], wk1t[0:1, 2048 - B:])
            nc.vector.tensor_copy(outT[:], dxT[:])
            nc.sync.dma_start(y_d[:], outT[:])
            return
        # V = Wk2.T @ [f|g]; out vt[j] = [128(h), (c,b)]
        for j in range(HB):
            for nb in range(NB):
                nc.tensor.matmul(
                    vt[:, j * 2 * B:(j + 1) * 2 * B],
                    wk2[:, nb * H + j * 128:nb * H + (j + 1) * 128],
                    fg8[:, nb * 2 * B:(nb + 1) * 2 * B],
                    start=(nb == 0), stop=(nb == NB - 1),
                )
        l2_mlp(kT_ps, w2k, 0, aTk)
        l2_bias_move(kT_sb, kT_ps, C_B2K)
        nc.vector.tensor_tensor(kk[:], kT_sb[:], kT_sb[:], ALU.mult)
        # ||k||^2 reduce + power chain (off the tail; PE slots these in
        # while waiting for the JVP inputs)
        for nb in range(NB):
            nc.tensor.matmul(red[0:1, 0:B], onescol[:, 0:1],
                             kk[:, nb * B:(nb + 1) * B],
                             start=(nb == 0), stop=(nb == NB - 1))
        nc.vector.tensor_copy(s2sb[:], red[0:1, 0:B])
        nc.vector.tensor_tensor(p4[:], s2sb[:], s2sb[:], ALU.mult)
        nc.vector.tensor_tensor(p8[:], p4[:], p4[:], ALU.mult)
        nc.vector.tensor_tensor(p16[:], p8[:], p8[:], ALU.mult)
        nc.vector.scalar_tensor_tensor(
            t1[:], p16[:], ALPHA * ALPHA, s2sb[:], ALU.mult, ALU.mult)
        nc.vector.scalar_tensor_tensor(
            t2[:], p8[:], BETA, s2sb[:], ALU.mult, ALU.mult)
        # svt = S * V   [128, (j, c, b)]
        sT_v = (sT[:].rearrange("p (j b) -> p j b", b=B)
                .unsqueeze(2).broadcast_to((128, HB, 2, B)))
        svt_v = svt[:].rearrange("p (j c b) -> p j c b", c=2, b=B)
        vt_v = vt[:, :].rearrange("p (j c b) -> p j c b", c=2, b=B)
        nc.vector.tensor_tensor(svt_v, vt_v, sT_v, ALU.mult)
        # J = Wk1.T @ svt; out jT[nb] = [128(n), (c,b)]
        for nb in range(NB):
            for j in range(HB):
                nc.tensor.matmul(
                    jT[:, nb * 2 * B:(nb + 1) * 2 * B],
                    wk1[:, j * N + nb * 128:j * N + nb * 128 + 128],
                    svt[:, j * 2 * B:(j + 1) * 2 * B],
                    start=(j == 0), stop=(j == HB - 1),
                )
        # Jf^2 and k*JG (read jT PSUM directly, strided on c)
        jT_v = jT[:, :].rearrange("p (nb c b) -> p nb c b", c=2, b=B)
        jfsq_v = jfsq[:].rearrange("p (nb b) -> p nb b", b=B)
        kjg_v = kjg[:].rearrange("p (nb b) -> p nb b", b=B)
        kT_v = kT_sb[:].rearrange("p (nb b) -> p nb b", b=B)
        nc.scalar.square(jfsq_v, jT_v[:, :, 0, :])
        nc.vector.tensor_tensor(kjg_v, kT_v, jT_v[:, :, 1, :], ALU.mult)
        for nb in range(NB):
            nc.tensor.matmul(red[0:1, B:2 * B], onescol[:, 0:1],
                             jfsq[:, nb * B:(nb + 1) * B],
                             start=(nb == 0), stop=(nb == NB - 1))
        for nb in range(NB):
            nc.tensor.matmul(red[0:1, 2 * B:3 * B], onescol[:, 0:1],
                             kjg[:, nb * B:(nb + 1) * B],
                             start=(nb == 0), stop=(nb == NB - 1))
        # mask chain
        nc.vector.tensor_tensor(m1[:], red[0:1, B:2 * B], t1[:], ALU.is_gt)
        nc.vector.scalar_tensor_tensor(
            m2[:], red[0:1, 2 * B:3 * B], EPS, t2[:], ALU.add, ALU.is_lt)
        nc.vector.tensor_tensor(mm[:], m1[:], m2[:], ALU.max)
        nc.vector.tensor_scalar(fac16[:], mm[:], -0.5, 1.0, ALU.mult, ALU.add)
        # fac broadcast to all partitions, then out = dx * fac
        nc.tensor.matmul(facBC[:, :], ones128[0:1, :], fac16[0:1, :],
                         start=True, stop=True)
        fbc_v = facBC[:, :].unsqueeze(1).broadcast_to((128, NB, B))
        outT_v = outT[:].rearrange("p (nb b) -> p nb b", b=B)
        dxT_v = dxT[:].rearrange("p (nb b) -> p nb b", b=B)
        nc.vector.tensor_tensor(outT_v, dxT_v, fbc_v, ALU.mult)

        nc.sync.dma_start(y_d[:], outT[:])


def build_module():
    nc = bass.Bass("TRN2", target_bir_lowering=False, debug=False,
                   num_devices=N_CORES)
    io = declare_io(nc)
    with PatchedTileContext(nc) as tc:
        emit_body(nc, tc, **io)
    return nc


def prep_inputs(t, x, Wf1, bf1, Wf2, bf2, Wg1, bg1, Wg2, bg2, Wk1, bk1, Wk2, bk2):
    """Host-side packing: returns per-core in_maps."""
    f16 = np.float16
    w1fg = _pack_l1(
        np.concatenate([np.asarray(Wf1).T, np.asarray(Wg1).T], axis=1), 16
    ).astype(f16)
    w2fg = np.concatenate(
        [_pack(np.ascontiguousarray(np.asarray(Wf2).T), HB),
         _pack(np.ascontiguousarray(np.asarray(Wg2).T), HB)], axis=1
    ).astype(f16)
    w1k = _pack_l1(np.ascontiguousarray(np.asarray(Wk1).T), HB).astype(NP8)
    w2k = _pack(np.ascontiguousarray(np.asarray(Wk2).T), HB).astype(NP8)
    wk2 = _pack(np.asarray(Wk2), NB).astype(NP8)
    wk1 = _pack(np.asarray(Wk1), HB).astype(NP8)
    cst = np.zeros((128, 64), f16)   # shared cols; x filled per core
    cst[:, C_B1F:C_B1F + 8] = np.asarray(bf1).reshape(8, 128).T
    cst[:, C_B1G:C_B1G + 8] = np.asarray(bg1).reshape(8, 128).T
    cst[:, C_B1K:C_B1K + 8] = np.asarray(bk1).reshape(8, 128).T
    cst[:, C_B2F:C_B2F + 2] = np.asarray(bf2).reshape(2, 128).T
    cst[:, C_B2G:C_B2G + 2] = np.asarray(bg2).reshape(2, 128).T
    cst[:, C_B2K:C_B2K + 2] = np.asarray(bk2).reshape(2, 128).T
    x = np.asarray(x, dtype=np.float32)
    shared = {
        "ka": np.concatenate([w1k, w2k, wk2], axis=1),
        "wk1": wk1,
    }
    wtail = np.concatenate([w1fg, w2fg], axis=1)
    in_maps = []
    for c in range(N_CORES):
        xT = _pack(np.ascontiguousarray(x[c * B:(c + 1) * B].T), NB)
        cstc = cst.copy()
        cstc[:, C_XT:C_XT + NB * B] = xT.astype(f16)
        in_maps.append({**shared,
                        "wfg": np.concatenate([cstc, wtail], axis=1)})
    return in_maps


def unshard_y(y_core):
    """[128, NB*B] transposed layout -> [B, N] sample-major."""
    return np.ascontiguousarray(
        np.asarray(y_core).reshape(128, NB, B).transpose(2, 1, 0)
        .reshape(B, N))


_CACHED_NC = None


def kernel(**inputs) -> np.ndarray:
    global _CACHED_NC
    if _CACHED_NC is None:
        _CACHED_NC = build_module()
    in_maps = prep_inputs(**{k: inputs[k] for k in (
        "t", "x", "Wf1", "bf1", "Wf2", "bf2", "Wg1", "bg1", "Wg2", "bg2",
        "Wk1", "bk1", "Wk2", "bk2")})
    res = run_bass_kernel_spmd(_CACHED_NC, in_maps, list(range(N_CORES)))
    return np.concatenate(
        [unshard_y(res.results[c]["y"]) for c in range(N_CORES)], axis=0
    ).astype(np.float32)


# revision 31
# speedup vs baseline: 1.0182x; 1.0182x over previous
"""Trainium2 Bass kernel for nn_ManifoldDynamic (v2, latency-optimized).

Math (per sample b):
    f = tanh(x@Wf1.T + bf1)@Wf2.T + bf2        (same for g, k)
    dx = f + g
    Jf = Wk1.T @ (S * (Wk2.T @ f)),  S = 1 - tanh(h_k)^2   (JVP, no Jacobian)
    c1 > EPS  <=>  ||Jf||^2 > 3600*(||k||^2)^9
    c2 < -EPS <=>  <k,JG> + EPS < 20*(||k||^2)^5
    out = dx * (1 - 0.5*mask)

Design notes (single-iteration latency is the metric; the timing harness
wraps the body in a For_i whose back edge has an all-engine barrier, so
iterations serialize and the body's serial latency is what counts):
  - All L2 matmuls emit TRANSPOSED outputs [n-part, (nblk, b)] so the JVP
    consumes them directly -- no PE transposes, no [16,256]-layout vector
    ops.  The output stays transposed; the host untransposes for free.
  - Sample-dim reductions (||k||^2, ||Jf||^2, <k,JG>) are ones-vector
    matmuls contracting the 128 partitions; the mask scalar chain runs on
    [1,16] tiles; fac is partition-broadcast by one more rank-1 matmul.
  - f/g weights stream in fp16 (2MB, accuracy-bound: fp8 gives 5e-2 >
    the 2e-2 gate); the whole k/JVP path is fp8 (1MB) -- the mask
    comparisons have ~1e10 of margin.
  - HW measurement showed DMA transfers serialize per core (~3ns/KB,
    ~340GB/s, independent of queue count) with ~1.1us fixed cost per
    transfer, so the whole weight set ships as THREE chunks: one 2MB
    f16 blob (biases | x.T | w1fg | w2fg) on SP, then w1k|w2k|wk2
    (fp8) and finally wk1 alone (fp8) on ACT -- V and svt hide under
    the wk1 transfer, so only the J matmuls + mask chain + output DMA
    trail the stream.  The tanh ACT table is preloaded by a dummy
    activation at iteration start.
"""

import numpy as np
import ml_dtypes

import concourse.bass as bass
import concourse.mybir as mybir
from concourse.tile import TileContext
from concourse.vector_clock import ScopedClock
from concourse.bass_utils import run_bass_kernel_spmd

N_CORES = 8
GP_COPY = True       # casts on gpsimd; False: DVE
LEVEL = 99           # phase truncation for profiling: 0=DMA only, 2=no JVP
BS, N, H = 128, 256, 1024
B = BS // N_CORES          # 16 rows per core
NB = N // 128              # 2 n-blocks
HB = H // 128              # 8 h-blocks
ALPHA, BETA, EPS = 60.0, 20.0, 1e-8

F16 = mybir.dt.float16
F32 = mybir.dt.float32
F8 = mybir.dt.float8e4
NP8 = ml_dtypes.float8_e4m3
ALU = mybir.AluOpType
ACTF = mybir.ActivationFunctionType

# bias tensor (f32, [128, 30]): col j = bias slice for region j, values on
# the partition axis (h-in-block for L1 slices, n-in-block for L2 slices)
C_B1F = 0            # b1f: cols 0-7
C_B1G = 8            # b1g: cols 8-15
C_B1K = 16           # b1k: cols 16-23
C_B2F = 24           # b2f: cols 24-25
C_B2G = 26           # b2g: cols 26-27
C_B2K = 28           # b2k: cols 28-29
C_XT = 30            # x.T packed, 32 cols f32 (cast on chip)
C_TOT = 62


class PatchedTileContext(TileContext):
    """walrus in this env rejects >1 sync wait per instruction; after
    scheduling, hoist excess waits onto same-engine NOPs placed directly
    before the instruction (same gating, one wait per instruction)."""

    _ws_counter = 0

    def _split_waits(self):
        import bass_rust as _br

        nc = self.nc
        for fn in nc.m.functions:
            for blk in fn.blocks:
                insts = list(blk.instructions)
                out = []
                changed = False
                for inst in insts:
                    si = inst.sync_info
                    if si is not None and si.on_wait and len(si.on_wait) > 1:
                        waits = list(si.on_wait)
                        del si.on_wait[:]
                        si.on_wait.append(waits[-1])
                        for w in waits[:-1]:
                            PatchedTileContext._ws_counter += 1
                            nop = _br.InstNoOp(
                                name=f"waitsplit_{PatchedTileContext._ws_counter}"
                            )
                            nop.engine = inst.engine
                            nop.sync_info = mybir.SyncInfo(
                                on_wait=[w], on_update=[])
                            nc.register_instruction(nop)
                            out.append(nop)
                        changed = True
                    out.append(inst)
                if changed:
                    blk.instructions = out

    def _drain_and_barrier(self, tick_clock, wait_clock):
        drain_inst = self.nc.sync.drain()
        wait_clock.add_sem_waits(
            drain_inst.ins, ScopedClock({None: tick_clock.global_clock})
        )
        self.nc.all_engine_barrier()
        assert self.sems is not None
        popped = self.nc._tile_sem_poison_stack.pop()
        assert popped is self._sem_poison
        self.nc.clear_and_free_semaphores(list(self.sems.allocated().values()))
        self.nc.all_engine_barrier()
        self._split_waits()
        # SWDGE descriptor-gen emits extended InstISA ops; inside a For_i
        # their .instr bytes are not populated by the default raw-bass
        # path -> "ISA wrong length" in codegen.  Lower them here.
        mybir.codegen_inst_isa_subclasses(self.nc)


def _pack(arr, pblk):
    """[pblk*128, F] -> [128, pblk*F]: partition-block p of the original
    lands at free-dim columns [p*F, (p+1)*F)."""
    k, f = arr.shape
    assert k == pblk * 128
    return np.ascontiguousarray(
        arr.reshape(pblk, 128, f).transpose(1, 0, 2).reshape(128, pblk * f)
    )


def _pack_l1(w1T, hblk):
    """[256, hblk*128] -> [128, hblk*2*128] as (hblk, kblk) [128,128]
    tiles: tile (i, k) at columns (i*2+k)*128."""
    return np.ascontiguousarray(
        w1T.reshape(NB, 128, hblk, 128).transpose(1, 2, 0, 3)
        .reshape(128, hblk * NB * 128)
    )


def declare_io(nc):
    def din(name, shape, dt):
        return nc.dram_tensor(name, shape, dt, kind="ExternalInput").ap()

    io = dict(
        wfg_d=din("wfg", [128, 64 + 8192 + 1024], F16),
        # ^ bias | x.T | w1fg | w2fg | w1k (fp8 bytes bitcast as f16 cols)
        ka_d=din("ka", [128, 4096], F8),       # w2k | wk2
        wk1_d=din("wk1", [128, 2048], F8),     # Wk1 natural (J), last chunk
        y_d=nc.dram_tensor("y", [128, NB * B], F32, kind="ExternalOutput").ap(),
    )
    return io


def emit_body(nc, tc, wfg_d, ka_d, wk1_d, y_d):
    with (
        tc.tile_pool(name="wpool", bufs=1) as wp,
        tc.tile_pool(name="apool", bufs=1) as ap,
        tc.tile_pool(name="psum", bufs=1, space="PSUM") as pp,
    ):
        # ---------------- SBUF tiles --------------------------------
        wfg = wp.tile([128, 64 + 8192 + 1024], F16, tag="wfg")
        ka = wp.tile([128, 4096], F8, tag="ka")
        wk1t = wp.tile([128, 2048], F8, tag="wk1")
        cst = wfg[:, 0:C_TOT]
        xt16 = wfg[:, C_XT:C_XT + NB * B]
        w1fg = wfg[:, 64:64 + 4096]
        w2fg = wfg[:, 64 + 4096:64 + 8192]
        w1k = wfg[:, 64 + 8192:64 + 9216].bitcast(F8)
        w2k = ka[:, 0:2048]
        wk2 = ka[:, 2048:4096]
        wk1 = wk1t[:, :]

        ones128 = ap.tile([1, 128], F16, tag="ones128")
        onescol = ap.tile([128, 1], F32, tag="onescol")
        tscr = ap.tile([1, B], F16, tag="tscr")
        xt8 = ap.tile([128, NB * B], F8, tag="xt8")
        aTf = ap.tile([128, HB * B], F16, tag="aTf")
        aTg = ap.tile([128, HB * B], F16, tag="aTg")
        aTk = ap.tile([128, HB * B], F8, tag="aTk")
        sq = ap.tile([128, HB * B], F32, tag="sq")
        sT = ap.tile([128, HB * B], F32, tag="sT")
        fT_sb = ap.tile([128, NB * B], F32, tag="fT_sb")
        gT_sb = ap.tile([128, NB * B], F32, tag="gT_sb")
        kT_sb = ap.tile([128, NB * B], F32, tag="kT_sb")
        fg8 = ap.tile([128, NB * 2 * B], F8, tag="fg8")   # (nb, c, b)
        svt = ap.tile([128, HB * 2 * B], F8, tag="svt")   # (j, c, b)
        dxT = ap.tile([128, NB * B], F32, tag="dxT")
        kk = ap.tile([128, NB * B], F32, tag="kk")
        jfsq = ap.tile([128, NB * B], F32, tag="jfsq")
        kjg = ap.tile([128, NB * B], F32, tag="kjg")
        s2sb = ap.tile([1, B], F32, tag="s2sb")
        p4 = ap.tile([1, B], F32, tag="p4")
        p8 = ap.tile([1, B], F32, tag="p8")
        p16 = ap.tile([1, B], F32, tag="p16")
        t1 = ap.tile([1, B], F32, tag="t1")
        t2 = ap.tile([1, B], F32, tag="t2")
        m1 = ap.tile([1, B], F32, tag="m1")
        m2 = ap.tile([1, B], F32, tag="m2")
        mm = ap.tile([1, B], F32, tag="mm")
        fac16 = ap.tile([1, B], F16, tag="fac16")
        outT = ap.tile([128, NB * B], F32, tag="outT")

        # ---------------- PSUM tiles (8 banks) ----------------------
        hTf = pp.tile([128, HB * B], F32, tag="hTf")
        hTg = pp.tile([128, HB * B], F32, tag="hTg")
        hTk = pp.tile([128, HB * B], F32, tag="hTk")
        vt = pp.tile([128, HB * 2 * B], F32, tag="vt")    # (j, c, b)
        l2ps = pp.tile([128, 3 * NB * B], F32, tag="l2ps")  # f | k | g
        jT = pp.tile([128, NB * 2 * B], F32, tag="jT")    # (nb, c, b)
        facBC = pp.tile([128, B], F32, tag="facBC")
        red = pp.tile([1, 3 * B], F32, tag="red")         # s2 | jf2 | c2dot

        fT_ps = l2ps[:, 0:NB * B]
        kT_ps = l2ps[:, NB * B:2 * NB * B]
        gT_ps = l2ps[:, 2 * NB * B:3 * NB * B]

        # ---------------- DMA issues --------------------------------
        # SP: x + consts (tiny), then the f weights.  DVE HWDGE carries
        # the g weights in parallel; ACT HWDGE the k forward weights;
        # Pool SWDGE the JVP weights.
        HALF1 = 16 * NB * 128 // 2
        HALF2 = HB * 2 * N // 2
        # Measured on HW: DMA transfers serialize chip-side (~3ns/KB
        # aggregate) and each chunk carries ~1.1us of fixed overhead, so
        # FEWER, BIGGER chunks win.  4 chunks, in compute order; the k
        # JVP weights land last (only J + mask + out trail the stream).
        nc.sync.dma_start(wfg[:], wfg_d[:])
        nc.scalar.dma_start(ka[:], ka_d[:])
        nc.scalar.dma_start(wk1t[:], wk1_d[:])
        gcopy = nc.gpsimd.tensor_copy if GP_COPY else nc.vector.tensor_copy
        nc.vector.memset(ones128[:], 1.0)
        nc.vector.memset(onescol[:], 1.0)
        gcopy(xt8[:], xt16)
        # Preload the tanh table set during the DMA stream: a throwaway
        # ACTIVATE pays the ~1.4us ACT_TABLE_LOAD off the critical path.
        nc.scalar.activation(tscr[:], ones128[0:1, 0:B], ACTF.Tanh)

        # ---------------- matmul helpers ----------------------------
        def l1_mlp(hT, w, xw, bc0):
            """L1 matmuls + in-place PSUM bias add (broadcast over b)."""
            for j in range(HB):
                for nk in range(NB):
                    nc.tensor.matmul(
                        hT[:, j * B:(j + 1) * B],
                        w[:, (j * NB + nk) * 128:(j * NB + nk + 1) * 128],
                        xw[:, nk * B:(nk + 1) * B],
                        start=(nk == 0), stop=(nk == NB - 1),
                    )
            hT_v = hT[:, :].rearrange("p (j b) -> p j b", b=B)
            b_bc = (cst[:, bc0:bc0 + HB].unsqueeze(2)
                    .broadcast_to((128, HB, B)))
            nc.vector.tensor_tensor(hT_v, hT_v, b_bc, ALU.add)

        def l2_mlp(ps, w, wcol, aT):
            for nb in range(NB):
                for j in range(HB):
                    nc.tensor.matmul(
                        ps[:, nb * B:(nb + 1) * B],
                        w[:, wcol + j * N + nb * 128:
                          wcol + j * N + nb * 128 + 128],
                        aT[:, j * B:(j + 1) * B],
                        start=(j == 0), stop=(j == HB - 1),
                    )

        def l2_bias_move(dst, ps, bc0):
            """PSUM -> SBUF with the L2 bias folded into the move."""
            dst_v = dst[:].rearrange("p (nb b) -> p nb b", b=B)
            ps_v = ps.rearrange("p (nb b) -> p nb b", b=B)
            b_bc = (cst[:, bc0:bc0 + NB].unsqueeze(2)
                    .broadcast_to((128, NB, B)))
            nc.vector.tensor_tensor(dst_v, ps_v, b_bc, ALU.add)

        # ============ interleaved program (emission order = dep order;
        # per-engine execution order is the subsequence per engine) =====
        if LEVEL < 1:
            # consumers force every DMA into the iteration
            for tl_ in (wfg, ka, wk1t):
                nc.vector.tensor_copy(outT[0:1, 0:B],
                                      tl_[0:1, tl_.shape[1] - B:])
            nc.vector.tensor_copy(outT[:], cst[:, 0:NB * B])
            nc.sync.dma_start(y_d[:], outT[:])
            return
        # f and g chains complete first (their weights arrive in chunk 2)
        fg8_v = fg8[:].rearrange("p (nb c b) -> p nb c b", c=2, b=B)
        l1_mlp(hTf, w1fg, xt16, C_B1F)
        nc.scalar.activation(aTf[:], hTf[:], ACTF.Tanh)
        l1_mlp(hTg, w1fg[:, HALF1:], xt16, C_B1G)
        nc.scalar.activation(aTg[:], hTg[:], ACTF.Tanh)
        # k L1 rides chunk 1 (w1k bitcast in wfg) -- its S chain is ready
        # long before the JVP needs it
        l1_mlp(hTk, w1k, xt8, C_B1K)
        nc.scalar.activation(aTk[:], hTk[:], ACTF.Tanh)
        nc.scalar.square(sq[:], aTk[:])
        nc.vector.tensor_scalar(sT[:], sq[:], -1.0, 1.0, ALU.mult, ALU.add)
        l2_mlp(fT_ps, w2fg, 0, aTf)
        l2_bias_move(fT_sb, fT_ps, C_B2F)
        gcopy(
            fg8_v[:, :, 0, :],
            fT_sb[:].rearrange("p (nb b) -> p nb b", b=B))
        l2_mlp(gT_ps, w2fg, HB * N, aTg)
        l2_bias_move(gT_sb, gT_ps, C_B2G)
        gcopy(
            fg8_v[:, :, 1, :],
            gT_sb[:].rearrange("p (nb b) -> p nb b", b=B))
        nc.vector.tensor_tensor(dxT[:], fT_sb[:], gT_sb[:], ALU.add)
        # k L2 (chunk ka)
        l2_mlp(kT_ps, w2k, 0, aTk)
        l2_bias_move(kT_sb, kT_ps, C_B2K)
        nc.vector.tensor_tensor(kk[:], kT_sb[:], kT_sb[:], ALU.mult)
        if LEVEL < 3:
            nc.vector.tensor_copy(outT[0:1, 0:B], wk1t[0:1, 2048 - B:])
            nc.vector.tensor_copy(outT[:], dxT[:])
            nc.sync.dma_start(y_d[:], outT[:])
            return
        # V = Wk2.T @ [f|g]; out vt[j] = [128(h), (c,b)]
        for j in range(HB):
            for nb in range(NB):
                nc.tensor.matmul(
                    vt[:, j * 2 * B:(j + 1) * 2 * B],
                    wk2[:, nb * H + j * 128:nb * H + (j + 1) * 128],
                    fg8[:, nb * 2 * B:(nb + 1) * 2 * B],
                    start=(nb == 0), stop=(nb == NB - 1),
                )
        l2_mlp(kT_ps, w2k, 0, aTk)
        l2_bias_move(kT_sb, kT_ps, C_B2K)
        nc.vector.tensor_tensor(kk[:], kT_sb[:], kT_sb[:], ALU.mult)
        # ||k||^2 reduce + power chain (off the tail; PE slots these in
        # while waiting for the JVP inputs)
        for nb in range(NB):
            nc.tensor.matmul(red[0:1, 0:B], onescol[:, 0:1],
                             kk[:, nb * B:(nb + 1) * B],
                             start=(nb == 0), stop=(nb == NB - 1))
        nc.vector.tensor_copy(s2sb[:], red[0:1, 0:B])
        nc.vector.tensor_tensor(p4[:], s2sb[:], s2sb[:], ALU.mult)
        nc.vector.tensor_tensor(p8[:], p4[:], p4[:], ALU.mult)
        nc.vector.tensor_tensor(p16[:], p8[:], p8[:], ALU.mult)
        nc.vector.scalar_tensor_tensor(
            t1[:], p16[:], ALPHA * ALPHA, s2sb[:], ALU.mult, ALU.mult)
        nc.vector.scalar_tensor_tensor(
            t2[:], p8[:], BETA, s2sb[:], ALU.mult, ALU.mult)
        # svt = S * V   [128, (j, c, b)]
        sT_v = (sT[:].rearrange("p (j b) -> p j b", b=B)
                .unsqueeze(2).broadcast_to((128, HB, 2, B)))
        svt_v = svt[:].rearrange("p (j c b) -> p j c b", c=2, b=B)
        vt_v = vt[:, :].rearrange("p (j c b) -> p j c b", c=2, b=B)
        nc.vector.tensor_tensor(svt_v, vt_v, sT_v, ALU.mult)
        # J = Wk1.T @ svt; out jT[nb] = [128(n), (c,b)]
        for nb in range(NB):
            for j in range(HB):
                nc.tensor.matmul(
                    jT[:, nb * 2 * B:(nb + 1) * 2 * B],
                    wk1[:, j * N + nb * 128:j * N + nb * 128 + 128],
                    svt[:, j * 2 * B:(j + 1) * 2 * B],
                    start=(j == 0), stop=(j == HB - 1),
                )
        # Jf^2 and k*JG (read jT PSUM directly, strided on c)
        jT_v = jT[:, :].rearrange("p (nb c b) -> p nb c b", c=2, b=B)
        jfsq_v = jfsq[:].rearrange("p (nb b) -> p nb b", b=B)
        kjg_v = kjg[:].rearrange("p (nb b) -> p nb b", b=B)
        kT_v = kT_sb[:].rearrange("p (nb b) -> p nb b", b=B)
        nc.scalar.square(jfsq_v, jT_v[:, :, 0, :])
        nc.vector.tensor_tensor(kjg_v, kT_v, jT_v[:, :, 1, :], ALU.mult)
        for nb in range(NB):
            nc.tensor.matmul(red[0:1, B:2 * B], onescol[:, 0:1],
                             jfsq[:, nb * B:(nb + 1) * B],
                             start=(nb == 0), stop=(nb == NB - 1))
        for nb in range(NB):
            nc.tensor.matmul(red[0:1, 2 * B:3 * B], onescol[:, 0:1],
                             kjg[:, nb * B:(nb + 1) * B],
                             start=(nb == 0), stop=(nb == NB - 1))
        # mask chain
        nc.vector.tensor_tensor(m1[:], red[0:1, B:2 * B], t1[:], ALU.is_gt)
        nc.vector.scalar_tensor_tensor(
            m2[:], red[0:1, 2 * B:3 * B], EPS, t2[:], ALU.add, ALU.is_lt)
        nc.vector.tensor_tensor(mm[:], m1[:], m2[:], ALU.max)
        nc.vector.tensor_scalar(fac16[:], mm[:], -0.5, 1.0, ALU.mult, ALU.add)
        # fac broadcast to all partitions, then out = dx * fac
        nc.tensor.matmul(facBC[:, :], ones128[0:1, :], fac16[0:1, :],
                         start=True, stop=True)
        fbc_v = facBC[:, :].unsqueeze(1).broadcast_to((128, NB, B))
        outT_v = outT[:].rearrange("p (nb b) -> p nb b", b=B)
        dxT_v = dxT[:].rearrange("p (nb b) -> p nb b", b=B)
        nc.vector.tensor_tensor(outT_v, dxT_v, fbc_v, ALU.mult)

        nc.sync.dma_start(y_d[:], outT[:])


def build_module():
    nc = bass.Bass("TRN2", target_bir_lowering=False, debug=False,
                   num_devices=N_CORES)
    io = declare_io(nc)
    with PatchedTileContext(nc) as tc:
        emit_body(nc, tc, **io)
    return nc


def prep_inputs(t, x, Wf1, bf1, Wf2, bf2, Wg1, bg1, Wg2, bg2, Wk1, bk1, Wk2, bk2):
    """Host-side packing: returns per-core in_maps."""
    f16 = np.float16
    w1fg = _pack_l1(
        np.concatenate([np.asarray(Wf1).T, np.asarray(Wg1).T], axis=1), 16
    ).astype(f16)
    w2fg = np.concatenate(
        [_pack(np.ascontiguousarray(np.asarray(Wf2).T), HB),
         _pack(np.ascontiguousarray(np.asarray(Wg2).T), HB)], axis=1
    ).astype(f16)
    w1k = _pack_l1(np.ascontiguousarray(np.asarray(Wk1).T), HB).astype(NP8)
    w2k = _pack(np.ascontiguousarray(np.asarray(Wk2).T), HB).astype(NP8)
    wk2 = _pack(np.asarray(Wk2), NB).astype(NP8)
    wk1 = _pack(np.asarray(Wk1), HB).astype(NP8)
    cst = np.zeros((128, 64), f16)   # shared cols; x filled per core
    cst[:, C_B1F:C_B1F + 8] = np.asarray(bf1).reshape(8, 128).T
    cst[:, C_B1G:C_B1G + 8] = np.asarray(bg1).reshape(8, 128).T
    cst[:, C_B1K:C_B1K + 8] = np.asarray(bk1).reshape(8, 128).T
    cst[:, C_B2F:C_B2F + 2] = np.asarray(bf2).reshape(2, 128).T
    cst[:, C_B2G:C_B2G + 2] = np.asarray(bg2).reshape(2, 128).T
    cst[:, C_B2K:C_B2K + 2] = np.asarray(bk2).reshape(2, 128).T
    x = np.asarray(x, dtype=np.float32)
    shared = {
        "ka": np.concatenate([w2k, wk2], axis=1),
        "wk1": wk1,
    }
    w1k16 = w1k.view(np.float16)           # fp8 bytes as f16 columns
    wtail = np.concatenate([w1fg, w2fg], axis=1)
    in_maps = []
    for c in range(N_CORES):
        xT = _pack(np.ascontiguousarray(x[c * B:(c + 1) * B].T), NB)
        cstc = cst.copy()
        cstc[:, C_XT:C_XT + NB * B] = xT.astype(f16)
        in_maps.append({**shared,
                        "wfg": np.concatenate([cstc, wtail, w1k16], axis=1)})
    return in_maps


def unshard_y(y_core):
    """[128, NB*B] transposed layout -> [B, N] sample-major."""
    return np.ascontiguousarray(
        np.asarray(y_core).reshape(128, NB, B).transpose(2, 1, 0)
        .reshape(B, N))


_CACHED_NC = None


def kernel(**inputs) -> np.ndarray:
    global _CACHED_NC
    if _CACHED_NC is None:
        _CACHED_NC = build_module()
    in_maps = prep_inputs(**{k: inputs[k] for k in (
        "t", "x", "Wf1", "bf1", "Wf2", "bf2", "Wg1", "bg1", "Wg2", "bg2",
        "Wk1", "bk1", "Wk2", "bk2")})
    res = run_bass_kernel_spmd(_CACHED_NC, in_maps, list(range(N_CORES)))
    return np.concatenate(
        [unshard_y(res.results[c]["y"]) for c in range(N_CORES)], axis=0
    ).astype(np.float32)


# revision 32
# speedup vs baseline: 1.1281x; 1.1080x over previous
"""Trainium2 Bass kernel for nn_ManifoldDynamic (v2, latency-optimized).

Math (per sample b):
    f = tanh(x@Wf1.T + bf1)@Wf2.T + bf2        (same for g, k)
    dx = f + g
    Jf = Wk1.T @ (S * (Wk2.T @ f)),  S = 1 - tanh(h_k)^2   (JVP, no Jacobian)
    c1 > EPS  <=>  ||Jf||^2 > 3600*(||k||^2)^9
    c2 < -EPS <=>  <k,JG> + EPS < 20*(||k||^2)^5
    out = dx * (1 - 0.5*mask)

Design notes (single-iteration latency is the metric; the timing harness
wraps the body in a For_i whose back edge has an all-engine barrier, so
iterations serialize and the body's serial latency is what counts):
  - All L2 matmuls emit TRANSPOSED outputs [n-part, (nblk, b)] so the JVP
    consumes them directly -- no PE transposes, no [16,256]-layout vector
    ops.  The output stays transposed; the host untransposes for free.
  - Sample-dim reductions (||k||^2, ||Jf||^2, <k,JG>) are ones-vector
    matmuls contracting the 128 partitions; the mask scalar chain runs on
    [1,16] tiles; fac is partition-broadcast by one more rank-1 matmul.
  - f/g weights stream in fp16 (2MB, accuracy-bound: fp8 gives 5e-2 >
    the 2e-2 gate); the whole k/JVP path is fp8 (1MB) -- the mask
    comparisons have ~1e10 of margin.
  - HW measurement showed DMA transfers serialize per core (~3ns/KB,
    ~340GB/s, independent of queue count) with ~1.1us fixed cost per
    transfer, so the whole weight set ships as THREE chunks: one 2MB
    f16 blob (biases | x.T | w1fg | w2fg) on SP, then w1k|w2k|wk2
    (fp8) and finally wk1 alone (fp8) on ACT -- V and svt hide under
    the wk1 transfer, so only the J matmuls + mask chain + output DMA
    trail the stream.  The tanh ACT table is preloaded by a dummy
    activation at iteration start.
"""

import numpy as np
import ml_dtypes

import concourse.bass as bass
import concourse.mybir as mybir
from concourse.tile import TileContext
from concourse.vector_clock import ScopedClock
from concourse.bass_utils import run_bass_kernel_spmd

N_CORES = 8
GP_COPY = True       # casts on gpsimd; False: DVE
LEVEL = 99           # phase truncation for profiling: 0=DMA only, 2=no JVP
BS, N, H = 128, 256, 1024
B = BS // N_CORES          # 16 rows per core
NB = N // 128              # 2 n-blocks
HB = H // 128              # 8 h-blocks
ALPHA, BETA, EPS = 60.0, 20.0, 1e-8

F16 = mybir.dt.float16
F32 = mybir.dt.float32
F8 = mybir.dt.float8e4
NP8 = ml_dtypes.float8_e4m3
ALU = mybir.AluOpType
ACTF = mybir.ActivationFunctionType

# bias tensor (f32, [128, 30]): col j = bias slice for region j, values on
# the partition axis (h-in-block for L1 slices, n-in-block for L2 slices)
C_B1F = 0            # b1f: cols 0-7
C_B1G = 8            # b1g: cols 8-15
C_B1K = 16           # b1k: cols 16-23
C_B2F = 24           # b2f: cols 24-25
C_B2G = 26           # b2g: cols 26-27
C_B2K = 28           # b2k: cols 28-29
C_XT = 30            # x.T packed, 32 cols f32 (cast on chip)
C_TOT = 62


class PatchedTileContext(TileContext):
    """walrus in this env rejects >1 sync wait per instruction; after
    scheduling, hoist excess waits onto same-engine NOPs placed directly
    before the instruction (same gating, one wait per instruction)."""

    _ws_counter = 0

    def _split_waits(self):
        import bass_rust as _br

        nc = self.nc
        for fn in nc.m.functions:
            for blk in fn.blocks:
                insts = list(blk.instructions)
                out = []
                changed = False
                for inst in insts:
                    si = inst.sync_info
                    if si is not None and si.on_wait and len(si.on_wait) > 1:
                        waits = list(si.on_wait)
                        del si.on_wait[:]
                        si.on_wait.append(waits[-1])
                        for w in waits[:-1]:
                            PatchedTileContext._ws_counter += 1
                            nop = _br.InstNoOp(
                                name=f"waitsplit_{PatchedTileContext._ws_counter}"
                            )
                            nop.engine = inst.engine
                            nop.sync_info = mybir.SyncInfo(
                                on_wait=[w], on_update=[])
                            nc.register_instruction(nop)
                            out.append(nop)
                        changed = True
                    out.append(inst)
                if changed:
                    blk.instructions = out

    def _drain_and_barrier(self, tick_clock, wait_clock):
        drain_inst = self.nc.sync.drain()
        wait_clock.add_sem_waits(
            drain_inst.ins, ScopedClock({None: tick_clock.global_clock})
        )
        self.nc.all_engine_barrier()
        assert self.sems is not None
        popped = self.nc._tile_sem_poison_stack.pop()
        assert popped is self._sem_poison
        self.nc.clear_and_free_semaphores(list(self.sems.allocated().values()))
        self.nc.all_engine_barrier()
        self._split_waits()
        # SWDGE descriptor-gen emits extended InstISA ops; inside a For_i
        # their .instr bytes are not populated by the default raw-bass
        # path -> "ISA wrong length" in codegen.  Lower them here.
        mybir.codegen_inst_isa_subclasses(self.nc)


def _pack(arr, pblk):
    """[pblk*128, F] -> [128, pblk*F]: partition-block p of the original
    lands at free-dim columns [p*F, (p+1)*F)."""
    k, f = arr.shape
    assert k == pblk * 128
    return np.ascontiguousarray(
        arr.reshape(pblk, 128, f).transpose(1, 0, 2).reshape(128, pblk * f)
    )


def _pack_l1(w1T, hblk):
    """[256, hblk*128] -> [128, hblk*2*128] as (hblk, kblk) [128,128]
    tiles: tile (i, k) at columns (i*2+k)*128."""
    return np.ascontiguousarray(
        w1T.reshape(NB, 128, hblk, 128).transpose(1, 2, 0, 3)
        .reshape(128, hblk * NB * 128)
    )


def declare_io(nc):
    def din(name, shape, dt):
        return nc.dram_tensor(name, shape, dt, kind="ExternalInput").ap()

    io = dict(
        wfg_d=din("wfg", [128, 64 + 8192], F16),  # bias|x.T|w1fg|w2fg
        ka_d=din("ka", [128, 6144], F8),       # w1k | w2k | wk2
        wk1_d=din("wk1", [128, 2048], F8),     # Wk1 natural (J), last chunk
        y_d=nc.dram_tensor("y", [128, NB * B], F32, kind="ExternalOutput").ap(),
    )
    return io


def emit_body(nc, tc, wfg_d, ka_d, wk1_d, y_d):
    with (
        tc.tile_pool(name="wpool", bufs=1) as wp,
        tc.tile_pool(name="apool", bufs=1) as ap,
        tc.tile_pool(name="psum", bufs=1, space="PSUM") as pp,
    ):
        # ---------------- SBUF tiles --------------------------------
        wfg = wp.tile([128, 64 + 8192], F16, tag="wfg")
        ka = wp.tile([128, 6144], F8, tag="ka")
        wk1t = wp.tile([128, 2048], F8, tag="wk1")
        cst = wfg[:, 0:C_TOT]
        xt16 = wfg[:, C_XT:C_XT + NB * B]
        w1fg = wfg[:, 64:64 + 4096]
        w2fg = wfg[:, 64 + 4096:64 + 8192]
        w1k = ka[:, 0:2048]
        w2k = ka[:, 2048:4096]
        wk2 = ka[:, 4096:6144]
        wk1 = wk1t[:, :]

        ones128 = ap.tile([1, 128], F16, tag="ones128")
        onescol = ap.tile([128, 1], F32, tag="onescol")
        tscr = ap.tile([1, B], F16, tag="tscr")
        xt8 = ap.tile([128, NB * B], F8, tag="xt8")
        aTf = ap.tile([128, HB * B], F16, tag="aTf")
        aTg = ap.tile([128, HB * B], F16, tag="aTg")
        aTk = ap.tile([128, HB * B], F8, tag="aTk")
        sq = ap.tile([128, HB * B], F32, tag="sq")
        sT = ap.tile([128, HB * B], F32, tag="sT")
        fT_sb = ap.tile([128, NB * B], F32, tag="fT_sb")
        gT_sb = ap.tile([128, NB * B], F32, tag="gT_sb")
        kT_sb = ap.tile([128, NB * B], F32, tag="kT_sb")
        fg8 = ap.tile([128, NB * 2 * B], F8, tag="fg8")   # (nb, c, b)
        svt = ap.tile([128, HB * 2 * B], F8, tag="svt")   # (j, c, b)
        dxT = ap.tile([128, NB * B], F32, tag="dxT")
        kk = ap.tile([128, NB * B], F32, tag="kk")
        jfsq = ap.tile([128, NB * B], F32, tag="jfsq")
        kjg = ap.tile([128, NB * B], F32, tag="kjg")
        s2sb = ap.tile([1, B], F32, tag="s2sb")
        p4 = ap.tile([1, B], F32, tag="p4")
        p8 = ap.tile([1, B], F32, tag="p8")
        p16 = ap.tile([1, B], F32, tag="p16")
        t1 = ap.tile([1, B], F32, tag="t1")
        t2 = ap.tile([1, B], F32, tag="t2")
        m1 = ap.tile([1, B], F32, tag="m1")
        m2 = ap.tile([1, B], F32, tag="m2")
        mm = ap.tile([1, B], F32, tag="mm")
        fac16 = ap.tile([1, B], F16, tag="fac16")
        outT = ap.tile([128, NB * B], F32, tag="outT")

        # ---------------- PSUM tiles (8 banks) ----------------------
        hTf = pp.tile([128, HB * B], F32, tag="hTf")
        hTg = pp.tile([128, HB * B], F32, tag="hTg")
        hTk = pp.tile([128, HB * B], F32, tag="hTk")
        vt = pp.tile([128, HB * 2 * B], F32, tag="vt")    # (j, c, b)
        l2ps = pp.tile([128, 3 * NB * B], F32, tag="l2ps")  # f | k | g
        jT = pp.tile([128, NB * 2 * B], F32, tag="jT")    # (nb, c, b)
        facBC = pp.tile([128, B], F32, tag="facBC")
        red = pp.tile([1, 3 * B], F32, tag="red")         # s2 | jf2 | c2dot

        fT_ps = l2ps[:, 0:NB * B]
        kT_ps = l2ps[:, NB * B:2 * NB * B]
        gT_ps = l2ps[:, 2 * NB * B:3 * NB * B]

        # ---------------- DMA issues --------------------------------
        # SP: x + consts (tiny), then the f weights.  DVE HWDGE carries
        # the g weights in parallel; ACT HWDGE the k forward weights;
        # Pool SWDGE the JVP weights.
        HALF1 = 16 * NB * 128 // 2
        HALF2 = HB * 2 * N // 2
        # Measured on HW: DMA transfers serialize chip-side (~3ns/KB
        # aggregate) and each chunk carries ~1.1us of fixed overhead, so
        # FEWER, BIGGER chunks win.  4 chunks, in compute order; the k
        # JVP weights land last (only J + mask + out trail the stream).
        nc.sync.dma_start(wfg[:], wfg_d[:])
        nc.scalar.dma_start(ka[:], ka_d[:])
        nc.scalar.dma_start(wk1t[:], wk1_d[:])
        gcopy = nc.gpsimd.tensor_copy if GP_COPY else nc.vector.tensor_copy
        nc.vector.memset(ones128[:], 1.0)
        nc.vector.memset(onescol[:], 1.0)
        gcopy(xt8[:], xt16)
        # Preload the tanh table set during the DMA stream: a throwaway
        # ACTIVATE pays the ~1.4us ACT_TABLE_LOAD off the critical path.
        nc.scalar.activation(tscr[:], ones128[0:1, 0:B], ACTF.Tanh)

        # ---------------- matmul helpers ----------------------------
        def l1_mlp(hT, w, xw, bc0):
            """L1 matmuls + in-place PSUM bias add (broadcast over b)."""
            for j in range(HB):
                for nk in range(NB):
                    nc.tensor.matmul(
                        hT[:, j * B:(j + 1) * B],
                        w[:, (j * NB + nk) * 128:(j * NB + nk + 1) * 128],
                        xw[:, nk * B:(nk + 1) * B],
                        start=(nk == 0), stop=(nk == NB - 1),
                    )
            hT_v = hT[:, :].rearrange("p (j b) -> p j b", b=B)
            b_bc = (cst[:, bc0:bc0 + HB].unsqueeze(2)
                    .broadcast_to((128, HB, B)))
            nc.vector.tensor_tensor(hT_v, hT_v, b_bc, ALU.add)

        def l2_mlp(ps, w, wcol, aT):
            for nb in range(NB):
                for j in range(HB):
                    nc.tensor.matmul(
                        ps[:, nb * B:(nb + 1) * B],
                        w[:, wcol + j * N + nb * 128:
                          wcol + j * N + nb * 128 + 128],
                        aT[:, j * B:(j + 1) * B],
                        start=(j == 0), stop=(j == HB - 1),
                    )

        def l2_bias_move(dst, ps, bc0):
            """PSUM -> SBUF with the L2 bias folded into the move."""
            dst_v = dst[:].rearrange("p (nb b) -> p nb b", b=B)
            ps_v = ps.rearrange("p (nb b) -> p nb b", b=B)
            b_bc = (cst[:, bc0:bc0 + NB].unsqueeze(2)
                    .broadcast_to((128, NB, B)))
            nc.vector.tensor_tensor(dst_v, ps_v, b_bc, ALU.add)

        # ============ interleaved program (emission order = dep order;
        # per-engine execution order is the subsequence per engine) =====
        if LEVEL < 1:
            # consumers force every DMA into the iteration
            for tl_ in (wfg, ka, wk1t):
                nc.vector.tensor_copy(outT[0:1, 0:B],
                                      tl_[0:1, tl_.shape[1] - B:])
            nc.vector.tensor_copy(outT[:], cst[:, 0:NB * B])
            nc.sync.dma_start(y_d[:], outT[:])
            return
        # f and g chains complete first (their weights arrive in chunk 2)
        fg8_v = fg8[:].rearrange("p (nb c b) -> p nb c b", c=2, b=B)
        l1_mlp(hTf, w1fg, xt16, C_B1F)
        nc.scalar.activation(aTf[:], hTf[:], ACTF.Tanh)
        l1_mlp(hTg, w1fg[:, HALF1:], xt16, C_B1G)
        nc.scalar.activation(aTg[:], hTg[:], ACTF.Tanh)
        l2_mlp(fT_ps, w2fg, 0, aTf)
        l2_bias_move(fT_sb, fT_ps, C_B2F)
        gcopy(
            fg8_v[:, :, 0, :],
            fT_sb[:].rearrange("p (nb b) -> p nb b", b=B))
        l2_mlp(gT_ps, w2fg, HB * N, aTg)
        l2_bias_move(gT_sb, gT_ps, C_B2G)
        gcopy(
            fg8_v[:, :, 1, :],
            gT_sb[:].rearrange("p (nb b) -> p nb b", b=B))
        nc.vector.tensor_tensor(dxT[:], fT_sb[:], gT_sb[:], ALU.add)
        # k chain (chunk ka); wk2 lands with the same chunk
        l1_mlp(hTk, w1k, xt8, C_B1K)
        nc.scalar.activation(aTk[:], hTk[:], ACTF.Tanh)
        nc.scalar.square(sq[:], aTk[:])
        nc.vector.tensor_scalar(sT[:], sq[:], -1.0, 1.0, ALU.mult, ALU.add)
        l2_mlp(kT_ps, w2k, 0, aTk)
        l2_bias_move(kT_sb, kT_ps, C_B2K)
        nc.vector.tensor_tensor(kk[:], kT_sb[:], kT_sb[:], ALU.mult)
        if LEVEL < 3:
            nc.vector.tensor_copy(outT[0:1, 0:B], wk1t[0:1, 2048 - B:])
            nc.vector.tensor_copy(outT[:], dxT[:])
            nc.sync.dma_start(y_d[:], outT[:])
            return
        # V = Wk2.T @ [f|g]; out vt[j] = [128(h), (c,b)]
        for j in range(HB):
            for nb in range(NB):
                nc.tensor.matmul(
                    vt[:, j * 2 * B:(j + 1) * 2 * B],
                    wk2[:, nb * H + j * 128:nb * H + (j + 1) * 128],
                    fg8[:, nb * 2 * B:(nb + 1) * 2 * B],
                    start=(nb == 0), stop=(nb == NB - 1),
                )
        l2_mlp(kT_ps, w2k, 0, aTk)
        l2_bias_move(kT_sb, kT_ps, C_B2K)
        nc.vector.tensor_tensor(kk[:], kT_sb[:], kT_sb[:], ALU.mult)
        # ||k||^2 reduce + power chain (off the tail; PE slots these in
        # while waiting for the JVP inputs)
        for nb in range(NB):
            nc.tensor.matmul(red[0:1, 0:B], onescol[:, 0:1],
                             kk[:, nb * B:(nb + 1) * B],
                             start=(nb == 0), stop=(nb == NB - 1))
        nc.vector.tensor_copy(s2sb[:], red[0:1, 0:B])
        nc.vector.tensor_tensor(p4[:], s2sb[:], s2sb[:], ALU.mult)
        nc.vector.tensor_tensor(p8[:], p4[:], p4[:], ALU.mult)
        nc.vector.tensor_tensor(p16[:], p8[:], p8[:], ALU.mult)
        nc.vector.scalar_tensor_tensor(
            t1[:], p16[:], ALPHA * ALPHA, s2sb[:], ALU.mult, ALU.mult)
        nc.vector.scalar_tensor_tensor(
            t2[:], p8[:], BETA, s2sb[:], ALU.mult, ALU.mult)
        # svt = S * V   [128, (j, c, b)]
        sT_v = (sT[:].rearrange("p (j b) -> p j b", b=B)
                .unsqueeze(2).broadcast_to((128, HB, 2, B)))
        svt_v = svt[:].rearrange("p (j c b) -> p j c b", c=2, b=B)
        vt_v = vt[:, :].rearrange("p (j c b) -> p j c b", c=2, b=B)
        nc.vector.tensor_tensor(svt_v, vt_v, sT_v, ALU.mult)
        # J = Wk1.T @ svt; out jT[nb] = [128(n), (c,b)]
        for nb in range(NB):
            for j in range(HB):
                nc.tensor.matmul(
                    jT[:, nb * 2 * B:(nb + 1) * 2 * B],
                    wk1[:, j * N + nb * 128:j * N + nb * 128 + 128],
                    svt[:, j * 2 * B:(j + 1) * 2 * B],
                    start=(j == 0), stop=(j == HB - 1),
                )
        # Jf^2 and k*JG (read jT PSUM directly, strided on c)
        jT_v = jT[:, :].rearrange("p (nb c b) -> p nb c b", c=2, b=B)
        jfsq_v = jfsq[:].rearrange("p (nb b) -> p nb b", b=B)
        kjg_v = kjg[:].rearrange("p (nb b) -> p nb b", b=B)
        kT_v = kT_sb[:].rearrange("p (nb b) -> p nb b", b=B)
        nc.scalar.square(jfsq_v, jT_v[:, :, 0, :])
        nc.vector.tensor_tensor(kjg_v, kT_v, jT_v[:, :, 1, :], ALU.mult)
        for nb in range(NB):
            nc.tensor.matmul(red[0:1, B:2 * B], onescol[:, 0:1],
                             jfsq[:, nb * B:(nb + 1) * B],
                             start=(nb == 0), stop=(nb == NB - 1))
        for nb in range(NB):
            nc.tensor.matmul(red[0:1, 2 * B:3 * B], onescol[:, 0:1],
                             kjg[:, nb * B:(nb + 1) * B],
                             start=(nb == 0), stop=(nb == NB - 1))
        # mask chain
        nc.vector.tensor_tensor(m1[:], red[0:1, B:2 * B], t1[:], ALU.is_gt)
        nc.vector.scalar_tensor_tensor(
            m2[:], red[0:1, 2 * B:3 * B], EPS, t2[:], ALU.add, ALU.is_lt)
        nc.vector.tensor_tensor(mm[:], m1[:], m2[:], ALU.max)
        nc.vector.tensor_scalar(fac16[:], mm[:], -0.5, 1.0, ALU.mult, ALU.add)
        # fac broadcast to all partitions, then out = dx * fac
        nc.tensor.matmul(facBC[:, :], ones128[0:1, :], fac16[0:1, :],
                         start=True, stop=True)
        fbc_v = facBC[:, :].unsqueeze(1).broadcast_to((128, NB, B))
        outT_v = outT[:].rearrange("p (nb b) -> p nb b", b=B)
        dxT_v = dxT[:].rearrange("p (nb b) -> p nb b", b=B)
        nc.vector.tensor_tensor(outT_v, dxT_v, fbc_v, ALU.mult)

        nc.sync.dma_start(y_d[:], outT[:])


def build_module():
    nc = bass.Bass("TRN2", target_bir_lowering=False, debug=False,
                   num_devices=N_CORES)
    io = declare_io(nc)
    with PatchedTileContext(nc) as tc:
        emit_body(nc, tc, **io)
    return nc


def prep_inputs(t, x, Wf1, bf1, Wf2, bf2, Wg1, bg1, Wg2, bg2, Wk1, bk1, Wk2, bk2):
    """Host-side packing: returns per-core in_maps."""
    f16 = np.float16
    w1fg = _pack_l1(
        np.concatenate([np.asarray(Wf1).T, np.asarray(Wg1).T], axis=1), 16
    ).astype(f16)
    w2fg = np.concatenate(
        [_pack(np.ascontiguousarray(np.asarray(Wf2).T), HB),
         _pack(np.ascontiguousarray(np.asarray(Wg2).T), HB)], axis=1
    ).astype(f16)
    w1k = _pack_l1(np.ascontiguousarray(np.asarray(Wk1).T), HB).astype(NP8)
    w2k = _pack(np.ascontiguousarray(np.asarray(Wk2).T), HB).astype(NP8)
    wk2 = _pack(np.asarray(Wk2), NB).astype(NP8)
    wk1 = _pack(np.asarray(Wk1), HB).astype(NP8)
    cst = np.zeros((128, 64), f16)   # shared cols; x filled per core
    cst[:, C_B1F:C_B1F + 8] = np.asarray(bf1).reshape(8, 128).T
    cst[:, C_B1G:C_B1G + 8] = np.asarray(bg1).reshape(8, 128).T
    cst[:, C_B1K:C_B1K + 8] = np.asarray(bk1).reshape(8, 128).T
    cst[:, C_B2F:C_B2F + 2] = np.asarray(bf2).reshape(2, 128).T
    cst[:, C_B2G:C_B2G + 2] = np.asarray(bg2).reshape(2, 128).T
    cst[:, C_B2K:C_B2K + 2] = np.asarray(bk2).reshape(2, 128).T
    x = np.asarray(x, dtype=np.float32)
    shared = {
        "ka": np.concatenate([w1k, w2k, wk2], axis=1),
        "wk1": wk1,
    }
    wtail = np.concatenate([w1fg, w2fg], axis=1)
    in_maps = []
    for c in range(N_CORES):
        xT = _pack(np.ascontiguousarray(x[c * B:(c + 1) * B].T), NB)
        cstc = cst.copy()
        cstc[:, C_XT:C_XT + NB * B] = xT.astype(f16)
        in_maps.append({**shared,
                        "wfg": np.concatenate([cstc, wtail], axis=1)})
    return in_maps


def unshard_y(y_core):
    """[128, NB*B] transposed layout -> [B, N] sample-major."""
    return np.ascontiguousarray(
        np.asarray(y_core).reshape(128, NB, B).transpose(2, 1, 0)
        .reshape(B, N))


_CACHED_NC = None


def kernel(**inputs) -> np.ndarray:
    global _CACHED_NC
    if _CACHED_NC is None:
        _CACHED_NC = build_module()
    in_maps = prep_inputs(**{k: inputs[k] for k in (
        "t", "x", "Wf1", "bf1", "Wf2", "bf2", "Wg1", "bg1", "Wg2", "bg2",
        "Wk1", "bk1", "Wk2", "bk2")})
    res = run_bass_kernel_spmd(_CACHED_NC, in_maps, list(range(N_CORES)))
    return np.concatenate(
        [unshard_y(res.results[c]["y"]) for c in range(N_CORES)], axis=0
    ).astype(np.float32)
